# revision 1
# baseline (speedup 1.0000x reference)
"""Grouped-expert FFN (MoE) kernel for Trainium2, expert-parallel over 8 NeuronCores.

v5 = v3 (bf16) + partial-fp8 GEMM2: the first KF2=1024 rows of the H=4096
contraction run as fp8e4 DoubleRow pair-blocks (2x PE rate on that slice),
the rest stays bf16 -- cuts GEMM2 PE cycles by 12.5% (6.25% total).

Scale plumbing: fp8 h stored at true scale (gelu output fits e4m3 range),
fp8 w2 at x2048; bf16 w2 also pre-scaled x2048 so both accumulate into the
same PSUM at scale 2048; the output activation applies 1/2048 + b2.
Measured exact rel err (CPU, deterministic inputs): 1.79e-2 < 2e-2.

  GEMM1: hiddenT[h,n] = w1[d,h].T @ xT[d,n]      (all bf16)
  GEMM2: outT[d,n]    = w2[h,d].T @ hiddenT[h,n] (fp8-DR head + bf16 tail)
"""

import numpy as np
import ml_dtypes

E_FULL = 16
N_TOK = 2048
D_DIM = 1024
H_DIM = 4096
N_CORES = 8
E_LOC = E_FULL // N_CORES  # 2 experts per core
NT = 1024                  # token half processed per phase (fits SBUF)
NB = 512                   # matmul moving-dim chunk (= one PSUM bank of fp32)

KF2 = 1024                 # H rows of GEMM2 contraction done in fp8-DR
KT8 = KF2 // 128           # 8  k-tiles in fp8
KP8 = KF2 // 256           # 4  DoubleRow pair-blocks
SW2 = 2048.0               # w2 scale (both fp8 and bf16 parts)

_CACHE = {}


def _build(bench_iters=None):
    from concourse import bass, tile, mybir, bacc
    from contextlib import nullcontext

    BF16 = mybir.dt.bfloat16
    FP8 = mybir.dt.float8e4
    F32 = mybir.dt.float32
    AF = mybir.ActivationFunctionType
    DR = mybir.MatmulPerfMode.DoubleRow

    nc = bacc.Bacc("TRN2", target_bir_lowering=False, debug=False)

    KD_ = D_DIM // 128
    KH_ = H_DIM // 128
    G_BF = (KH_ - KT8) // 8  # 3 bf16 groups of 8 k-tiles
    xT = nc.dram_tensor("xT", (E_LOC, D_DIM, N_TOK), BF16, kind="ExternalInput").ap()
    # host-swizzled: w1s[e, m, p, k*128+j] = w1[e, k*128+p, m*128+j]
    w1 = nc.dram_tensor(
        "w1s", (E_LOC, KH_, 128, KD_ * 128), BF16, kind="ExternalInput"
    ).ap()
    # fp8 head of w2 (x2048), paired for DoubleRow, partition dim FIRST so the
    # [128, KP8, 2, 128] SBUF tile DMA is a same-order copy:
    #   w2p8[e, m2, p, c, i, j] = w2[e, c*256 + i*128 + p, m2*128 + j] * 2048
    w2p8 = nc.dram_tensor(
        "w2p8", (E_LOC, KD_, 128, KP8, 2, 128), FP8, kind="ExternalInput"
    ).ap()
    # bf16 tail of w2 (x2048): w2s[e, m2, g, p, ki*128+j] =
    #   w2[e, KF2 + (g*8+ki)*128 + p, m2*128+j] * 2048
    w2 = nc.dram_tensor(
        "w2s", (E_LOC, KD_, G_BF, 128, 8 * 128), BF16, kind="ExternalInput"
    ).ap()
    b1c = nc.dram_tensor("b1c", (128, H_DIM // 128), F32, kind="ExternalInput").ap()
    b2c = nc.dram_tensor("b2c", (128, D_DIM // 128), F32, kind="ExternalInput").ap()
    outT = nc.dram_tensor("outT", (E_LOC, D_DIM, N_TOK), F32, kind="ExternalOutput").ap()

    KD = D_DIM // 128   # 8  k-tiles for GEMM1
    KH = H_DIM // 128   # 32 k-tiles for GEMM2
    MH = H_DIM // 128   # 32 m-tiles (hidden rows) for GEMM1
    MD = D_DIM // 128   # 8  m-tiles (out rows) for GEMM2
    NBS = NT // NB      # 2 moving chunks per phase

    with tile.TileContext(nc) as tc:
        with (
            tc.tile_pool(name="xp", bufs=8) as xp,
            tc.tile_pool(name="hp", bufs=MH - KT8) as hp,
            tc.tile_pool(name="hp8", bufs=KP8) as hp8,
            tc.tile_pool(name="w1p", bufs=4) as w1p,
            tc.tile_pool(name="w2p", bufs=4) as w2p,
            tc.tile_pool(name="w2p8p", bufs=2) as w2p8p,
            tc.tile_pool(name="op", bufs=2) as op,
            tc.tile_pool(name="tsp", bufs=4) as tsp,
            tc.tile_pool(name="bp", bufs=1) as bp,
            tc.tile_pool(name="ps", bufs=8, space=bass.MemorySpace.PSUM) as ps,
        ):
            loop_cm = (
                tc.For_i(
                    0,
                    bench_iters,
                    1,
                    hint_engines=(
                        mybir.EngineType.PE,
                        mybir.EngineType.Activation,
                        mybir.EngineType.SP,
                        mybir.EngineType.DVE,
                        mybir.EngineType.Pool,
                    ),
                )
                if bench_iters is not None
                else nullcontext()
            )
            with loop_cm:
              # ---- PE warmup: dummy matmuls on memset data (no DMA deps) ----
              wut = bp.tile([128, 128], BF16, tag="wu")
              wux = bp.tile([128, NB], BF16, tag="wux")
              nc.vector.memset(wut[:], 0.0)
              nc.vector.memset(wux[:], 0.0)
              wup = ps.tile([128, NB], F32, tag="ps", name="wup")
              for _ in range(20):
                  nc.tensor.matmul(wup[:], wut[:], wux[:], start=True, stop=True)

              # first phase's first weight block + first x tile go FIRST so the
              # real matmuls can start ~2us in instead of ~17us.
              wblk0 = w1p.tile([128, KD * 128], BF16, tag="w1", name="wblk")
              nc.sync.dma_start(wblk0[:], w1[0, 0])

              b1t = bp.tile([128, H_DIM // 128], F32, tag="b1")
              b2t = bp.tile([128, D_DIM // 128], F32, tag="b2")

              for e in range(E_LOC):
                for t in range(N_TOK // NT):
                    first = e == 0 and t == 0
                    # ---- load xT half: 8 tiles [128d, NT] ----
                    xts = []
                    for k in range(KD):
                        xt = xp.tile([128, NT], BF16, tag="x")
                        nc.sync.dma_start(
                            xt[:],
                            xT[e, k * 128 : (k + 1) * 128, t * NT : (t + 1) * NT],
                        )
                        xts.append(xt)
                        if first and k == 0:
                            nc.sync.dma_start(b1t[:], b1c[:])
                            nc.sync.dma_start(b2t[:], b2c[:])

                    # ---- phase A: hiddenT[h, nt] = gelu(w1.T @ xT + b1) ----
                    # m < KT8 -> fp8 pair tiles (consumed by GEMM2's DR head);
                    # m >= KT8 -> bf16 tiles.
                    hts = {}
                    h8s = []
                    for m in range(MH):
                        pa = [ps.tile([128, NB], F32, tag="ps", name=f"pa{_}") for _ in range(NBS)]
                        if first and m == 0:
                            wblk = wblk0
                        else:
                            wblk = w1p.tile([128, KD * 128], BF16, tag="w1", name="wblk")
                            nc.sync.dma_start(wblk[:], w1[e, m])
                        for k in range(KD):
                            for nb in range(NBS):
                                nc.tensor.matmul(
                                    pa[nb][:],
                                    wblk[:, k * 128 : (k + 1) * 128],
                                    xts[k][:, nb * NB : (nb + 1) * NB],
                                    start=(k == 0),
                                    stop=(k == KD - 1),
                                )
                        if m < KT8:
                            if m % 2 == 0:
                                h8 = hp8.tile([128, 2, NT], FP8, tag="h8")
                                h8s.append(h8)
                            dst = h8s[m // 2]
                            for nb in range(NBS):
                                nc.scalar.activation(
                                    dst[:, m % 2, nb * NB : (nb + 1) * NB],
                                    pa[nb][:],
                                    AF.Gelu,
                                    bias=b1t[:, m : m + 1],
                                )
                        else:
                            ht = hp.tile([128, NT], BF16, tag="h")
                            for nb in range(NBS):
                                nc.scalar.activation(
                                    ht[:, nb * NB : (nb + 1) * NB],
                                    pa[nb][:],
                                    AF.Gelu,
                                    bias=b1t[:, m : m + 1],
                                )
                            hts[m] = ht

                    # ---- phase B: outT[d, nt] = (w2.T @ hiddenT)/SW2 + b2 ----
                    for m2 in range(MD):
                        # fp8-DR head accumulates in its OWN psum tiles (pd);
                        # bf16 tail in pb. Mixed-perf-mode matmuls sharing one
                        # PSUM accumulation group misaccumulate on HW, so the
                        # two groups are combined with a DVE add instead.
                        pd = [ps.tile([128, NB], F32, tag="ps", name=f"pd{_}") for _ in range(NBS)]
                        pb = [ps.tile([128, NB], F32, tag="ps", name=f"pb{_}") for _ in range(NBS)]
                        # fp8-DR head: KP8 pair-blocks, one small DMA
                        w8t = w2p8p.tile([128, KP8, 2, 128], FP8, tag="w28", name="w8t")
                        nc.sync.dma_start(w8t[:], w2p8[e, m2])
                        for c in range(KP8):
                            for nb in range(NBS):
                                nc.tensor.matmul(
                                    pd[nb][:],
                                    w8t[:, c],
                                    h8s[c][:, :, nb * NB : (nb + 1) * NB],
                                    start=(c == 0),
                                    stop=(c == KP8 - 1),
                                    perf_mode=DR,
                                )
                        # bf16 tail
                        for g in range(G_BF):
                            wblk2 = w2p.tile([128, 8 * 128], BF16, tag="w2", name="wblk2")
                            nc.sync.dma_start(wblk2[:], w2[e, m2, g])
                            for ki in range(8):
                                k = KT8 + g * 8 + ki
                                for nb in range(NBS):
                                    nc.tensor.matmul(
                                        pb[nb][:],
                                        wblk2[:, ki * 128 : (ki + 1) * 128],
                                        hts[k][:, nb * NB : (nb + 1) * NB],
                                        start=(g == 0 and ki == 0),
                                        stop=(k == KH - 1),
                                    )
                        ot = op.tile([128, NT], F32, tag="o")
                        for nb in range(NBS):
                            # ScalarE evacuates the DR head (PSUM->SBUF); the
                            # DVE add then has only ONE PSUM operand (walrus
                            # rejects InstTensorTensor with two PSUM inputs).
                            hsum = tsp.tile([128, NB], F32, tag="hs", name="hsum")
                            nc.scalar.copy(hsum[:], pd[nb][:])
                            ts = tsp.tile([128, NB], F32, tag="ts", name="ts")
                            nc.vector.tensor_add(ts[:], pb[nb][:], hsum[:])
                            nc.scalar.activation(
                                ot[:, nb * NB : (nb + 1) * NB],
                                ts[:],
                                AF.Identity,
                                bias=b2t[:, m2 : m2 + 1],
                                scale=1.0 / SW2,
                            )
                            nc.sync.dma_start(
                                outT[
                                    e,
                                    m2 * 128 : (m2 + 1) * 128,
                                    t * NT + nb * NB : t * NT + (nb + 1) * NB,
                                ],
                                ot[:, nb * NB : (nb + 1) * NB],
                            )

    nc.compile()
    return nc


def get_nc():
    if "nc" not in _CACHE:
        _CACHE["nc"] = _build()
    return _CACHE["nc"]


def _bf16(x):
    return np.asarray(x, dtype=np.float32).astype(ml_dtypes.bfloat16)


def _fp8(x, s):
    v = np.clip(np.asarray(x, np.float32) * s, -240.0, 240.0)
    return v.astype(ml_dtypes.float8_e4m3)


def _swizzle_w1(w1_loc):
    # [E, D, H] -> [E, MH, 128p, KD*128] with w1s[e,m,p,k*128+j] = w1[e,k*128+p,m*128+j]
    e = w1_loc.shape[0]
    v = w1_loc.reshape(e, D_DIM // 128, 128, H_DIM // 128, 128)  # e,k,p,m,j
    return np.ascontiguousarray(v.transpose(0, 3, 2, 1, 4)).reshape(
        e, H_DIM // 128, 128, (D_DIM // 128) * 128
    )


def _swizzle_w2_bf(w2_loc):
    # bf16 tail (H rows >= KF2), x2048:
    # [E, H-KF2, D] -> [E, MD, G_BF, 128p, 8*128]
    e = w2_loc.shape[0]
    g_bf = (H_DIM - KF2) // 1024
    v = w2_loc.reshape(e, g_bf, 8, 128, D_DIM // 128, 128)  # e,g,ki,p,m2,j
    return np.ascontiguousarray(v.transpose(0, 4, 1, 3, 2, 5)).reshape(
        e, D_DIM // 128, g_bf, 128, 8 * 128
    )


def _swizzle_w2_fp8(w2_head):
    # fp8 head (H rows < KF2), paired: [E, KF2, D] -> [E, MD, 128, KP8, 2, 128]
    e = w2_head.shape[0]
    v = w2_head.reshape(e, KP8, 2, 128, D_DIM // 128, 128)  # e,c,i,p,m2,j
    return np.ascontiguousarray(v.transpose(0, 4, 3, 1, 2, 5))


def make_in_maps(x, w1, w2, b1, b2):
    b1c = np.ascontiguousarray(b1.reshape(H_DIM // 128, 128).T, dtype=np.float32)
    b2c = np.ascontiguousarray(b2.reshape(D_DIM // 128, 128).T, dtype=np.float32)
    in_maps = []
    for c in range(N_CORES):
        sl = slice(E_LOC * c, E_LOC * (c + 1))
        w2_loc = np.asarray(w2)[sl]
        in_maps.append(
            {
                "xT": _bf16(np.asarray(x)[sl].transpose(0, 2, 1)),
                "w1s": _bf16(_swizzle_w1(np.asarray(w1)[sl])),
                "w2p8": _fp8(_swizzle_w2_fp8(w2_loc[:, :KF2] ), SW2),
                "w2s": _bf16(_swizzle_w2_bf(w2_loc[:, KF2:] * SW2)),
                "b1c": b1c,
                "b2c": b2c,
            }
        )
    return in_maps


def kernel(x, w1, w2, b1, b2):
    from concourse import bass_utils

    nc = get_nc()
    in_maps = make_in_maps(x, w1, w2, b1, b2)
    res = bass_utils.run_bass_kernel_spmd(nc, in_maps, core_ids=list(range(N_CORES)))
    out = np.empty((E_FULL, N_TOK, D_DIM), dtype=np.float32)
    for c in range(N_CORES):
        out[E_LOC * c : E_LOC * (c + 1)] = res.results[c]["outT"].transpose(0, 2, 1)
    return out



# revision 3
# speedup vs baseline: 1.0362x; 1.0362x over previous
"""Grouped-expert FFN (MoE) kernel for Trainium2, expert-parallel over 8 NeuronCores.

v5 = v3 (bf16) + partial-fp8 GEMM2: the first KF2=1024 rows of the H=4096
contraction run as fp8e4 DoubleRow pair-blocks (2x PE rate on that slice),
the rest stays bf16 -- cuts GEMM2 PE cycles by 12.5% (6.25% total).

Scale plumbing: fp8 h stored at true scale (gelu output fits e4m3 range),
fp8 w2 at x2048; bf16 w2 also pre-scaled x2048 so both accumulate into the
same PSUM at scale 2048; the output activation applies 1/2048 + b2.
Measured exact rel err (CPU, deterministic inputs): 1.79e-2 < 2e-2.

  GEMM1: hiddenT[h,n] = w1[d,h].T @ xT[d,n]      (all bf16)
  GEMM2: outT[d,n]    = w2[h,d].T @ hiddenT[h,n] (fp8-DR head + bf16 tail)
"""

import numpy as np
import ml_dtypes

E_FULL = 16
N_TOK = 2048
D_DIM = 1024
H_DIM = 4096
N_CORES = 8
E_LOC = E_FULL // N_CORES  # 2 experts per core
NT = 1024                  # token half processed per phase (fits SBUF)
NB = 512                   # matmul moving-dim chunk (= one PSUM bank of fp32)

KF2 = 1024                 # H rows of GEMM2 contraction done in fp8-DR
KT8 = KF2 // 128           # 8  k-tiles in fp8
KP8 = KF2 // 256           # 4  DoubleRow pair-blocks
SW2 = 2048.0               # w2 scale (both fp8 and bf16 parts)

_CACHE = {}


def _dedup_ldweights(nc):
    """Drop redundant InstLdweights: when consecutive LDWs on the PE stream
    load the identical stationary AP (with only non-self-loading matmuls in
    between and no sync attached to the duplicate), the PE array already
    holds the weights -- the reload only serializes with its matmul (~40ns
    per MM measured).  Runs after TileContext exit, before nc.compile()."""
    from concourse import mybir

    PE = mybir.EngineType.PE
    n = 0
    for blk in nc.m.functions[0].blocks:
        new = []
        last_key = None
        for inst in blk.instructions:
            if getattr(inst, "engine", None) != PE:
                new.append(inst)
                continue
            tn = type(inst).__name__
            if tn == "InstLdweights":
                si = inst.sync_info
                has_sync = si is not None and (
                    len(si.on_wait) > 0 or len(si.on_update) > 0
                )
                key = (
                    str(inst.ins[0]),
                    str(inst.perf_mode),
                    str(inst.is_transpose),
                    str(inst.tile_position),
                    str(inst.tile_size),
                )
                if key == last_key and not has_sync:
                    n += 1
                    continue
                last_key = key
                new.append(inst)
            elif tn == "InstMatmult" and inst.ldweights is False:
                new.append(inst)
            else:
                last_key = None
                new.append(inst)
        blk.instructions[:] = new
    return n


def _build(bench_iters=None):
    from concourse import bass, tile, mybir, bacc
    from contextlib import nullcontext

    BF16 = mybir.dt.bfloat16
    FP8 = mybir.dt.float8e4
    F32 = mybir.dt.float32
    AF = mybir.ActivationFunctionType
    DR = mybir.MatmulPerfMode.DoubleRow

    nc = bacc.Bacc("TRN2", target_bir_lowering=False, debug=False)

    KD_ = D_DIM // 128
    KH_ = H_DIM // 128
    G_BF = (KH_ - KT8) // 8  # 3 bf16 groups of 8 k-tiles
    xT = nc.dram_tensor("xT", (E_LOC, D_DIM, N_TOK), BF16, kind="ExternalInput").ap()
    # host-swizzled: w1s[e, m, p, k*128+j] = w1[e, k*128+p, m*128+j]
    w1 = nc.dram_tensor(
        "w1s", (E_LOC, KH_, 128, KD_ * 128), BF16, kind="ExternalInput"
    ).ap()
    # fp8 head of w2 (x2048), paired for DoubleRow, partition dim FIRST so the
    # [128, KP8, 2, 128] SBUF tile DMA is a same-order copy:
    #   w2p8[e, m2, p, c, i, j] = w2[e, c*256 + i*128 + p, m2*128 + j] * 2048
    w2p8 = nc.dram_tensor(
        "w2p8", (E_LOC, KD_, 128, KP8, 2, 128), FP8, kind="ExternalInput"
    ).ap()
    # bf16 tail of w2 (x2048): w2s[e, m2, g, p, ki*128+j] =
    #   w2[e, KF2 + (g*8+ki)*128 + p, m2*128+j] * 2048
    w2 = nc.dram_tensor(
        "w2s", (E_LOC, KD_, G_BF, 128, 8 * 128), BF16, kind="ExternalInput"
    ).ap()
    b1c = nc.dram_tensor("b1c", (128, H_DIM // 128), F32, kind="ExternalInput").ap()
    b2c = nc.dram_tensor("b2c", (128, D_DIM // 128), F32, kind="ExternalInput").ap()
    outT = nc.dram_tensor("outT", (E_LOC, D_DIM, N_TOK), F32, kind="ExternalOutput").ap()

    KD = D_DIM // 128   # 8  k-tiles for GEMM1
    KH = H_DIM // 128   # 32 k-tiles for GEMM2
    MH = H_DIM // 128   # 32 m-tiles (hidden rows) for GEMM1
    MD = D_DIM // 128   # 8  m-tiles (out rows) for GEMM2
    NBS = NT // NB      # 2 moving chunks per phase

    with tile.TileContext(nc) as tc:
        with (
            tc.tile_pool(name="xp", bufs=8) as xp,
            tc.tile_pool(name="hp", bufs=MH - KT8) as hp,
            tc.tile_pool(name="hp8", bufs=KP8) as hp8,
            tc.tile_pool(name="w1p", bufs=4) as w1p,
            tc.tile_pool(name="w2p", bufs=4) as w2p,
            tc.tile_pool(name="w2p8p", bufs=2) as w2p8p,
            tc.tile_pool(name="op", bufs=2) as op,
            tc.tile_pool(name="tsp", bufs=4) as tsp,
            tc.tile_pool(name="bp", bufs=1) as bp,
            tc.tile_pool(name="ps", bufs=8, space=bass.MemorySpace.PSUM) as ps,
        ):
            loop_cm = (
                tc.For_i(
                    0,
                    bench_iters,
                    1,
                    hint_engines=(
                        mybir.EngineType.PE,
                        mybir.EngineType.Activation,
                        mybir.EngineType.SP,
                        mybir.EngineType.DVE,
                        mybir.EngineType.Pool,
                    ),
                )
                if bench_iters is not None
                else nullcontext()
            )
            with loop_cm:
              # ---- PE warmup: dummy matmuls on memset data (no DMA deps) ----
              wut = bp.tile([128, 128], BF16, tag="wu")
              wux = bp.tile([128, NB], BF16, tag="wux")
              nc.vector.memset(wut[:], 0.0)
              nc.vector.memset(wux[:], 0.0)
              wup = ps.tile([128, NB], F32, tag="ps", name="wup")
              for _ in range(20):
                  nc.tensor.matmul(wup[:], wut[:], wux[:], start=True, stop=True)

              # first phase's first weight block + first x tile go FIRST so the
              # real matmuls can start ~2us in instead of ~17us.
              wblk0 = w1p.tile([128, KD * 128], BF16, tag="w1", name="wblk")
              nc.sync.dma_start(wblk0[:], w1[0, 0])

              b1t = bp.tile([128, H_DIM // 128], F32, tag="b1")
              b2t = bp.tile([128, D_DIM // 128], F32, tag="b2")

              for e in range(E_LOC):
                for t in range(N_TOK // NT):
                    first = e == 0 and t == 0
                    # ---- load xT half: 8 tiles [128d, NT] ----
                    xts = []
                    for k in range(KD):
                        xt = xp.tile([128, NT], BF16, tag="x")
                        nc.sync.dma_start(
                            xt[:],
                            xT[e, k * 128 : (k + 1) * 128, t * NT : (t + 1) * NT],
                        )
                        xts.append(xt)
                        if first and k == 0:
                            nc.sync.dma_start(b1t[:], b1c[:])
                            nc.sync.dma_start(b2t[:], b2c[:])

                    # ---- phase A: hiddenT[h, nt] = gelu(w1.T @ xT + b1) ----
                    # m < KT8 -> fp8 pair tiles (consumed by GEMM2's DR head);
                    # m >= KT8 -> bf16 tiles.
                    hts = {}
                    h8s = []
                    for m in range(MH):
                        pa = [ps.tile([128, NB], F32, tag="ps", name=f"pa{_}") for _ in range(NBS)]
                        if first and m == 0:
                            wblk = wblk0
                        else:
                            wblk = w1p.tile([128, KD * 128], BF16, tag="w1", name="wblk")
                            nc.sync.dma_start(wblk[:], w1[e, m])
                        for k in range(KD):
                            for nb in range(NBS):
                                nc.tensor.matmul(
                                    pa[nb][:],
                                    wblk[:, k * 128 : (k + 1) * 128],
                                    xts[k][:, nb * NB : (nb + 1) * NB],
                                    start=(k == 0),
                                    stop=(k == KD - 1),
                                )
                        if m < KT8:
                            if m % 2 == 0:
                                h8 = hp8.tile([128, 2, NT], FP8, tag="h8")
                                h8s.append(h8)
                            dst = h8s[m // 2]
                            for nb in range(NBS):
                                nc.scalar.activation(
                                    dst[:, m % 2, nb * NB : (nb + 1) * NB],
                                    pa[nb][:],
                                    AF.Gelu,
                                    bias=b1t[:, m : m + 1],
                                )
                        else:
                            ht = hp.tile([128, NT], BF16, tag="h")
                            for nb in range(NBS):
                                nc.scalar.activation(
                                    ht[:, nb * NB : (nb + 1) * NB],
                                    pa[nb][:],
                                    AF.Gelu,
                                    bias=b1t[:, m : m + 1],
                                )
                            hts[m] = ht

                    # ---- phase B: outT[d, nt] = (w2.T @ hiddenT)/SW2 + b2 ----
                    for m2 in range(MD):
                        # fp8-DR head accumulates in its OWN psum tiles (pd);
                        # bf16 tail in pb. Mixed-perf-mode matmuls sharing one
                        # PSUM accumulation group misaccumulate on HW, so the
                        # two groups are combined with a DVE add instead.
                        pd = [ps.tile([128, NB], F32, tag="ps", name=f"pd{_}") for _ in range(NBS)]
                        pb = [ps.tile([128, NB], F32, tag="ps", name=f"pb{_}") for _ in range(NBS)]
                        # fp8-DR head: KP8 pair-blocks, one small DMA
                        w8t = w2p8p.tile([128, KP8, 2, 128], FP8, tag="w28", name="w8t")
                        nc.sync.dma_start(w8t[:], w2p8[e, m2])
                        for c in range(KP8):
                            for nb in range(NBS):
                                nc.tensor.matmul(
                                    pd[nb][:],
                                    w8t[:, c],
                                    h8s[c][:, :, nb * NB : (nb + 1) * NB],
                                    start=(c == 0),
                                    stop=(c == KP8 - 1),
                                    perf_mode=DR,
                                )
                        # bf16 tail
                        for g in range(G_BF):
                            wblk2 = w2p.tile([128, 8 * 128], BF16, tag="w2", name="wblk2")
                            nc.sync.dma_start(wblk2[:], w2[e, m2, g])
                            for ki in range(8):
                                k = KT8 + g * 8 + ki
                                for nb in range(NBS):
                                    nc.tensor.matmul(
                                        pb[nb][:],
                                        wblk2[:, ki * 128 : (ki + 1) * 128],
                                        hts[k][:, nb * NB : (nb + 1) * NB],
                                        start=(g == 0 and ki == 0),
                                        stop=(k == KH - 1),
                                    )
                        ot = op.tile([128, NT], F32, tag="o")
                        for nb in range(NBS):
                            # ScalarE evacuates the DR head (PSUM->SBUF); the
                            # DVE add then has only ONE PSUM operand (walrus
                            # rejects InstTensorTensor with two PSUM inputs).
                            hsum = tsp.tile([128, NB], F32, tag="hs", name="hsum")
                            nc.scalar.copy(hsum[:], pd[nb][:])
                            ts = tsp.tile([128, NB], F32, tag="ts", name="ts")
                            nc.vector.tensor_add(ts[:], pb[nb][:], hsum[:])
                            nc.scalar.activation(
                                ot[:, nb * NB : (nb + 1) * NB],
                                ts[:],
                                AF.Identity,
                                bias=b2t[:, m2 : m2 + 1],
                                scale=1.0 / SW2,
                            )
                            nc.sync.dma_start(
                                outT[
                                    e,
                                    m2 * 128 : (m2 + 1) * 128,
                                    t * NT + nb * NB : t * NT + (nb + 1) * NB,
                                ],
                                ot[:, nb * NB : (nb + 1) * NB],
                            )

    _dedup_ldweights(nc)
    nc.compile()
    return nc


def get_nc():
    if "nc" not in _CACHE:
        _CACHE["nc"] = _build()
    return _CACHE["nc"]


def _bf16(x):
    return np.asarray(x, dtype=np.float32).astype(ml_dtypes.bfloat16)


def _fp8(x, s):
    v = np.clip(np.asarray(x, np.float32) * s, -240.0, 240.0)
    return v.astype(ml_dtypes.float8_e4m3)


def _swizzle_w1(w1_loc):
    # [E, D, H] -> [E, MH, 128p, KD*128] with w1s[e,m,p,k*128+j] = w1[e,k*128+p,m*128+j]
    e = w1_loc.shape[0]
    v = w1_loc.reshape(e, D_DIM // 128, 128, H_DIM // 128, 128)  # e,k,p,m,j
    return np.ascontiguousarray(v.transpose(0, 3, 2, 1, 4)).reshape(
        e, H_DIM // 128, 128, (D_DIM // 128) * 128
    )


def _swizzle_w2_bf(w2_loc):
    # bf16 tail (H rows >= KF2), x2048:
    # [E, H-KF2, D] -> [E, MD, G_BF, 128p, 8*128]
    e = w2_loc.shape[0]
    g_bf = (H_DIM - KF2) // 1024
    v = w2_loc.reshape(e, g_bf, 8, 128, D_DIM // 128, 128)  # e,g,ki,p,m2,j
    return np.ascontiguousarray(v.transpose(0, 4, 1, 3, 2, 5)).reshape(
        e, D_DIM // 128, g_bf, 128, 8 * 128
    )


def _swizzle_w2_fp8(w2_head):
    # fp8 head (H rows < KF2), paired: [E, KF2, D] -> [E, MD, 128, KP8, 2, 128]
    e = w2_head.shape[0]
    v = w2_head.reshape(e, KP8, 2, 128, D_DIM // 128, 128)  # e,c,i,p,m2,j
    return np.ascontiguousarray(v.transpose(0, 4, 3, 1, 2, 5))


def make_in_maps(x, w1, w2, b1, b2):
    b1c = np.ascontiguousarray(b1.reshape(H_DIM // 128, 128).T, dtype=np.float32)
    b2c = np.ascontiguousarray(b2.reshape(D_DIM // 128, 128).T, dtype=np.float32)
    in_maps = []
    for c in range(N_CORES):
        sl = slice(E_LOC * c, E_LOC * (c + 1))
        w2_loc = np.asarray(w2)[sl]
        in_maps.append(
            {
                "xT": _bf16(np.asarray(x)[sl].transpose(0, 2, 1)),
                "w1s": _bf16(_swizzle_w1(np.asarray(w1)[sl])),
                "w2p8": _fp8(_swizzle_w2_fp8(w2_loc[:, :KF2] ), SW2),
                "w2s": _bf16(_swizzle_w2_bf(w2_loc[:, KF2:] * SW2)),
                "b1c": b1c,
                "b2c": b2c,
            }
        )
    return in_maps


def kernel(x, w1, w2, b1, b2):
    from concourse import bass_utils

    nc = get_nc()
    in_maps = make_in_maps(x, w1, w2, b1, b2)
    res = bass_utils.run_bass_kernel_spmd(nc, in_maps, core_ids=list(range(N_CORES)))
    out = np.empty((E_FULL, N_TOK, D_DIM), dtype=np.float32)
    for c in range(N_CORES):
        out[E_LOC * c : E_LOC * (c + 1)] = res.results[c]["outT"].transpose(0, 2, 1)
    return out

